# revision 19
# baseline (speedup 1.0000x reference)
"""Diagonal SSM kernel for Trainium2 (8 NeuronCores, batch-parallel).

Computes, for x [8, 4096, 1024], W_decay/W_input [1024, 1024], biases [1024]:
    decays     = sigmoid(x @ W_decay.T + b_decay)
    injections = x @ W_input.T + b_input
    states_t   = decays_t * states_{t-1} + injections_t      (scan over T)

Sharding: batch b -> core b (8 batches, 8 cores, no collectives).

Layout strategy: all transposes/casts happen HOST-side (numpy), so the
device program is a pure matmul->activation->scan pipeline with zero PE
transposes:
  - host passes xT [D, T] and W.T [d, e] per core; output is written as
    yT [D, T] fp32 and transposed back on the host,
  - decay projection in fp8-e4m3 DoubleRow (scaled x*16, W*2048; the
    sigmoid's activation scale undoes 1/32768): 4 virtual-K=256 matmuls
    per [128-channel x 512-step] tile,
  - injection projection in bf16 (fp8 there fails the 2e-2 gate:
    injection errors feed the scan directly; measured 3e-2),
  - sigmoid(z*s + b_decay) on the scalar engine out of PSUM; injection
    bias-add on the vector engine (tensor_scalar_add) out of PSUM,
  - recurrence: native DVE tensor_tensor_scan per [128 ch x 512 steps]
    fp32 tile, chained across panels via its `initial` operand,
  - weight/x DMAs spread across engine queues; a dozen junk matmuls
    during the load phase warm the PE HAM clock-gate.

PE stream: 8 panels x (32 DoubleRow + 64 bf16) MMs ~ 166 us; scalar
(~44 us), DVE (~115 us) and DMA hide under it.  Numerics (numpy sim of
the exact quantization): rel err 1.30e-2 vs gate 2e-2.
"""

import sys

if "/opt/trn_rl_repo" not in sys.path:
    sys.path.insert(0, "/opt/trn_rl_repo")

from contextlib import ExitStack

import numpy as np
import ml_dtypes

import concourse.bass as bass  # noqa: F401
import concourse.tile as tile
from concourse import bacc, mybir
from concourse.bass_utils import run_bass_kernel_spmd

N_CORES = 8
B, T, D, P = 8, 4096, 1024, 128
PANEL = 512                  # time-panel width (one PSUM bank of fp32)
N_PANELS = T // PANEL        # 8
EB = D // P                  # 8 output-channel blocks
DB = D // P                  # 8 bf16 contraction blocks
DB2 = D // (2 * P)           # 4 fp8 DoubleRow contraction blocks

F32 = mybir.dt.float32
BF16 = mybir.dt.bfloat16
FP8 = mybir.dt.float8e4

SX = 16.0                    # fp8 scale on x
SW = 2048.0                  # fp8 scale on W_decay
DEC_FP8 = True
WARM_MMS = 16

_cached_nc = {}


def _build():
    key = ("nc", DEC_FP8)
    if key in _cached_nc:
        return _cached_nc[key]

    nc = bacc.Bacc(
        "TRN2",
        target_bir_lowering=False,
        debug=False,
        enable_asserts=True,
        num_devices=N_CORES,
    )

    # host-prepped layouts (see run()):
    #   xt  [D, T]      bf16   x transposed
    #   xq  [D//2, 2T]  fp8    pair-interleaved fp8 view of xT (decay MMs)
    #   wdq [D//2, 2D]  fp8    pair-interleaved W_decay.T * SW
    #   wit [D, D]      bf16   W_input.T
    #   yt  [D, T]      f32    output, transposed back on host
    x_ap = nc.dram_tensor("xt", [D, T], BF16, kind="ExternalInput").ap()
    wi_ap = nc.dram_tensor("wit", [D, D], BF16, kind="ExternalInput").ap()
    if DEC_FP8:
        xq_ap = nc.dram_tensor(
            "xq", [DB2 * P, N_PANELS * 2 * PANEL], FP8, kind="ExternalInput"
        ).ap()
        wd_ap = nc.dram_tensor(
            "wdq", [DB2 * P, 2 * D], FP8, kind="ExternalInput").ap()
    else:
        wd_ap = nc.dram_tensor("wdt", [D, D], BF16, kind="ExternalInput").ap()
    bd_ap = nc.dram_tensor("bd", [D], F32, kind="ExternalInput").ap()
    bi_ap = nc.dram_tensor("bi", [D], F32, kind="ExternalInput").ap()
    y_ap = nc.dram_tensor("yt", [D, T], F32, kind="ExternalOutput").ap()

    with tile.TileContext(nc) as tc, ExitStack() as ctx:
        singles = ctx.enter_context(tc.tile_pool(name="singles", bufs=1))

        # ---- PE warm-up while weights/x stream in: junk matmuls keep the
        # HAM activity window busy so the first real MMs run at 2.4 GHz.
        scratch = singles.tile([P, PANEL], BF16, tag="scratch")
        nc.vector.memset(scratch[:], 0)

        psum_mm = ctx.enter_context(
            tc.tile_pool(name="psum_mm", bufs=4, space="PSUM"))
        warm = psum_mm.tile([P, PANEL], F32, tag="pzd")
        for w in range(WARM_MMS):
            nc.tensor.matmul(
                warm[:, 0:2 * P],
                scratch[:, 0:P],
                scratch[:, 0:2 * P],
                start=(w == 0),
                stop=(w == WARM_MMS - 1),
            )

        # ---- bias / weight loads, spread across engine queues; weights
        # land in two column-halves so eb 0-3 can start before the full
        # matrices arrive ----
        bd_sb = singles.tile([P, EB], F32, tag="bd")
        bi_sb = singles.tile([P, EB], F32, tag="bi")

        wt_pool = ctx.enter_context(tc.tile_pool(name="wt", bufs=1))
        wdT = {}
        wiT = {}
        H = D // 2
        assert DEC_FP8
        for db2 in range(DB2):
            wdT[db2] = wt_pool.tile(
                [P, 2, D], FP8, tag=f"wdq{db2}", name=f"wdq{db2}")
        wd3 = {
            db2: wd_ap[db2 * P:(db2 + 1) * P, :].rearrange(
                "p (two e) -> p two e", two=2)
            for db2 in range(DB2)
        }
        for db in range(DB):
            wiT[db] = wt_pool.tile([P, D], BF16, tag=f"wit{db}", name=f"wit{db}")
        # supply order follows the eb consumption order; sync and scalar
        # queues each carry half the tiles so the first eb group is fed
        # within a few us
        for half in range(2):
            cs = slice(half * H, (half + 1) * H)
            for db2 in range(DB2):
                eng = nc.sync if db2 % 2 == 0 else nc.scalar
                eng.dma_start(wdT[db2][:, :, cs], wd3[db2][:, :, cs])
            for db in range(DB):
                eng = nc.sync if db % 2 == 0 else nc.scalar
                eng.dma_start(
                    wiT[db][:, cs], wi_ap[db * P:(db + 1) * P, cs])

        # biases only needed at the first activation; keep them off the
        # weight-feed queues
        nc.gpsimd.dma_start(bd_sb[:], bd_ap.rearrange("(f p) -> p f", p=P))
        nc.gpsimd.dma_start(bi_sb[:], bi_ap.rearrange("(f p) -> p f", p=P))

        xt_pool = ctx.enter_context(tc.tile_pool(name="xt", bufs=2))
        dec_pool = ctx.enter_context(tc.tile_pool(name="dec", bufs=6))
        st_pool = ctx.enter_context(tc.tile_pool(name="st", bufs=2))

        def load_panel(p):
            """Issue the x-tile DMAs for panel p."""
            xq = []
            if DEC_FP8:
                for db2 in range(DB2):
                    t_ = xt_pool.tile([P, 2, PANEL], FP8, tag=f"xq{db2}")
                    nc.gpsimd.dma_start(
                        t_[:],
                        xq_ap[db2 * P:(db2 + 1) * P,
                              p * 2 * PANEL:(p + 1) * 2 * PANEL].rearrange(
                                  "p (two n) -> p two n", two=2),
                    )
                    xq.append(t_)
            xt = []
            for db in range(DB):
                t_ = xt_pool.tile([P, PANEL], BF16, tag=f"xt{db}")
                # spread the x supply across all three DMA rings
                eng = (nc.gpsimd, nc.gpsimd, nc.gpsimd, nc.gpsimd,
                       nc.sync, nc.sync, nc.scalar, nc.scalar)[db]
                eng.dma_start(
                    t_[:], x_ap[db * P:(db + 1) * P, p * PANEL:(p + 1) * PANEL])
                xt.append(t_)
            return xq, xt

        prev_st = [None] * EB
        xq, xt = load_panel(0)
        for p in range(N_PANELS):
            nxt = None
            for eb in range(EB):
                pzd = psum_mm.tile([P, PANEL], F32, tag="pzd")
                if DEC_FP8:
                    for db2 in range(DB2):
                        nc.tensor.matmul(
                            pzd[:],
                            wdT[db2][:, :, eb * P:(eb + 1) * P],
                            xq[db2][:],
                            start=(db2 == 0),
                            stop=(db2 == DB2 - 1),
                            perf_mode=mybir.MatmulPerfMode.DoubleRow,
                        )
                else:
                    for db in range(DB):
                        nc.tensor.matmul(
                            pzd[:],
                            wdT[db][:, eb * P:(eb + 1) * P],
                            xt[db][:],
                            start=(db == 0),
                            stop=(db == DB - 1),
                        )
                pzi = psum_mm.tile([P, PANEL], F32, tag="pzi")
                for db in range(DB):
                    nc.tensor.matmul(
                        pzi[:],
                        wiT[db][:, eb * P:(eb + 1) * P],
                        xt[db][:],
                        start=(db == 0),
                        stop=(db == DB - 1),
                    )

                dec = dec_pool.tile([P, PANEL], F32, tag="dec")
                nc.scalar.activation(
                    dec[:],
                    pzd[:],
                    mybir.ActivationFunctionType.Sigmoid,
                    bias=bd_sb[:, eb:eb + 1],
                    scale=(1.0 / (SX * SW)) if DEC_FP8 else 1.0,
                )
                # injection bias-add on the vector engine, in parallel
                # with the sigmoid on scalar
                inj = dec_pool.tile([P, PANEL], F32, tag="inj")
                nc.vector.tensor_scalar_add(inj[:], pzi[:], bi_sb[:, eb:eb + 1])

                st = st_pool.tile([P, PANEL], F32, tag=f"st{eb}")
                init = 0.0 if p == 0 else prev_st[eb][:, PANEL - 1:PANEL]
                nc.vector.tensor_tensor_scan(
                    st[:],
                    dec[:],
                    inj[:],
                    init,
                    mybir.AluOpType.mult,
                    mybir.AluOpType.add,
                )
                prev_st[eb] = st

                # stores rotate across the three DMA-capable engines;
                # the final two tiles split column-wise across queues so
                # the post-matmul tail is short parallel transfers
                row = y_ap[eb * P:(eb + 1) * P, p * PANEL:(p + 1) * PANEL]
                if p == N_PANELS - 1 and eb >= EB - 2:
                    n_sp = 4 if eb == EB - 1 else 2
                    cw = PANEL // n_sp
                    for si in range(n_sp):
                        eng = (nc.sync, nc.scalar, nc.gpsimd, nc.scalar)[si]
                        eng.dma_start(
                            row[:, si * cw:(si + 1) * cw],
                            st[:, si * cw:(si + 1) * cw],
                        )
                else:
                    eng = (nc.sync, nc.scalar, nc.gpsimd)[eb % 3]
                    eng.dma_start(row, st[:])

                # prefetch next panel mid-way through this one
                if eb == 3 and p + 1 < N_PANELS:
                    nxt = load_panel(p + 1)

            if nxt is not None:
                xq, xt = nxt

    nc.compile()
    _cached_nc[key] = nc
    return nc


def run(inputs: dict, trace: bool = False, tmpdir: str | None = None):
    """Run on 8 cores; returns (output [8, T, D], BassKernelResults)."""
    nc = _build()
    x = np.asarray(inputs["x_seq"], dtype=np.float32)
    wd = np.asarray(inputs["W_decay"], dtype=np.float32)
    bd = np.ascontiguousarray(np.asarray(inputs["b_decay"], dtype=np.float32))
    wi = np.asarray(inputs["W_input"], dtype=np.float32)
    bi = np.ascontiguousarray(np.asarray(inputs["b_input"], dtype=np.float32))
    # host-side layout prep: transpose + casts
    bf16 = ml_dtypes.bfloat16
    fp8 = ml_dtypes.float8_e4m3
    wiT = np.ascontiguousarray(wi.T).astype(bf16)

    def pairs_w(a):
        # [d, e] -> [d//256 * 128, 2*e]: row = db2*128 + p holds the pair
        # (d = db2*256 + p, d = db2*256 + 128 + p) blocks side by side
        d, e = a.shape
        return np.ascontiguousarray(
            a.reshape(DB2, 2, P, e).transpose(0, 2, 1, 3).reshape(DB2 * P, 2 * e)
        )

    if DEC_FP8:
        wd8 = pairs_w(
            np.clip(np.ascontiguousarray(wd.T) * np.float32(SW), -240, 240
                    ).astype(fp8).astype(np.float32)
        ).astype(fp8)
    else:
        wdT = np.ascontiguousarray(wd.T).astype(bf16)

    in_maps = []
    for b in range(N_CORES):
        xT = np.ascontiguousarray(x[b].T)
        m = {
            "xt": xT.astype(bf16),
            "wit": wiT,
            "bd": bd,
            "bi": bi,
        }
        if DEC_FP8:
            x8 = np.clip(xT * np.float32(SX), -240, 240).astype(fp8)
            # [d, t] -> [d//256*128, panels*2*512]: per row pair-block cols
            m["xq"] = np.ascontiguousarray(
                x8.reshape(DB2, 2, P, N_PANELS, PANEL)
                .transpose(0, 2, 3, 1, 4)
                .reshape(DB2 * P, N_PANELS * 2 * PANEL)
            )
            m["wdq"] = wd8
        else:
            m["wdt"] = wdT
        in_maps.append(m)

    res = run_bass_kernel_spmd(
        nc, in_maps, core_ids=list(range(N_CORES)), trace=trace, tmpdir=tmpdir
    )
    out = np.stack(
        [res.results[b]["yt"].T for b in range(N_CORES)], axis=0
    )
    return np.ascontiguousarray(out), res


def kernel(x_seq, W_decay, b_decay, W_input, b_input) -> np.ndarray:
    out, _ = run(
        {
            "x_seq": x_seq,
            "W_decay": W_decay,
            "b_decay": b_decay,
            "W_input": W_input,
            "b_input": b_input,
        }
    )
    return out
